# revision 1
# baseline (speedup 1.0000x reference)
"""GQA (no RoPE) Trainium2 kernel, 8 NeuronCores.

Sharding: 2 batches x 4 group-pair shards (2 KV groups + their 8 query heads
per core). All projections computed locally from pre-transposed bf16 inputs;
attention in transposed (key-major) layout so softmax denominators fall out of
the attn@v matmul via an appended ones-column on V; AllGather of normalized
attention outputs within each batch's 4-core group; o_proj column-sharded
(no all-reduce needed).

Self-contained: hardcodes shapes B=2, S=1024, D=2048, G=8, HG=4, HD=64.
"""

import os
import sys

sys.path.insert(0, "/opt/trn_rl_repo")

import numpy as np
import ml_dtypes

import concourse.bass as bass
import concourse.mybir as mybir
import concourse.tile as tile
from concourse import bacc
from concourse import bass_utils

BF16 = mybir.dt.bfloat16
F32 = mybir.dt.float32
AF = mybir.ActivationFunctionType

B, S, D = 2, 1024, 2048
G, HG, HD = 8, 4, 64            # groups, heads/group, head dim
P = 128                          # partitions
NCORES = 8
GPC = 2                          # groups per core
CQ = GPC * HG * HD               # q channels per core = 512
CK = GPC * HD                    # k/v channels per core = 128
CO = D // 4                      # output cols per core = 512
DC = D // P                      # contract chunks = 16
SC = S // P                      # seq chunks = 8
SEG = 512                        # psum bank width in f32
AG_CHUNKS = 4                    # 1 = single AllGather, 4 = per q-block


def _build_nc():
    nc = bacc.Bacc(
        "TRN2",
        target_bir_lowering=False,
        debug=False,
        enable_asserts=False,
        num_devices=NCORES,
    )

    # ---- I/O ----
    qt = nc.dram_tensor("qt", [D, S], BF16, kind="ExternalInput").ap()
    kt = nc.dram_tensor("kt", [D, S], BF16, kind="ExternalInput").ap()
    vt = nc.dram_tensor("vt", [D, S], BF16, kind="ExternalInput").ap()
    wqt = nc.dram_tensor("wqt", [D, CQ], BF16, kind="ExternalInput").ap()
    wkt = nc.dram_tensor("wkt", [D, CK], BF16, kind="ExternalInput").ap()
    wvt = nc.dram_tensor("wvt", [D, CK], BF16, kind="ExternalInput").ap()
    wot = nc.dram_tensor("wot", [D, CO], BF16, kind="ExternalInput").ap()
    bo = nc.dram_tensor("bo", [1, CO], BF16, kind="ExternalInput").ap()
    tri = nc.dram_tensor("tri", [P, P], BF16, kind="ExternalInput").ap()
    out = nc.dram_tensor("out", [S, CO], F32, kind="ExternalOutput").ap()

    with tile.TileContext(nc) as tc:
        with (
            tc.tile_pool(name="consts", bufs=1) as cp,
            tc.tile_pool(name="res", bufs=1) as rp,
            tc.tile_pool(name="psA", bufs=2, space="PSUM") as psA,
            tc.tile_pool(name="psB", bufs=2, space="PSUM") as psB,
            tc.tile_pool(name="dram", bufs=1, space="DRAM") as dp,
        ):
            tri_sb = cp.tile([P, P], BF16)
            nc.sync.dma_start(tri_sb[:], tri[:])
            bo_sb = cp.tile([1, CO], BF16)
            nc.sync.dma_start(bo_sb[:], bo[:])
            ones_sb = cp.tile([1, P], BF16)
            nc.vector.memset(ones_sb[:], 1.0)

            # resident projection outputs; head-major with partition base 0
            # so every scores matmul sees lhsT/rhs at the same base partition
            qt_sb = rp.tile([HD, GPC * HG, S], BF16)   # q^T per head
            kt_sb = rp.tile([HD, GPC, S], BF16)        # k^T per group
            vaug = rp.tile([P, SC, GPC, HD + 1], BF16)  # v natural + ones col
            attn_sb = rp.tile([P, CQ // P, S], BF16)   # normalized attn^T local

            nc.vector.memset(vaug[:, :, :, HD:HD + 1], 1.0)

            # ---- load transposed activations & weights, d-chunked ----
            with tc.tile_pool(name="xt", bufs=1) as xp:
                kx = [xp.tile([P, S], BF16, name=f"kx{d}") for d in range(DC)]
                wk = [xp.tile([P, CK], BF16, name=f"wk{d}") for d in range(DC)]
                vx = [xp.tile([P, S], BF16, name=f"vx{d}") for d in range(DC)]
                wv = [xp.tile([P, CK], BF16, name=f"wv{d}") for d in range(DC)]
                qx = [xp.tile([P, S], BF16, name=f"qx{d}") for d in range(DC)]
                wq = [xp.tile([P, CQ], BF16, name=f"wq{d}") for d in range(DC)]
                for d in range(DC):
                    r = slice(d * P, (d + 1) * P)
                    nc.sync.dma_start(kx[d][:], kt[r, :])
                    nc.sync.dma_start(wk[d][:], wkt[r, :])
                for d in range(DC):
                    r = slice(d * P, (d + 1) * P)
                    nc.sync.dma_start(vx[d][:], vt[r, :])
                    nc.sync.dma_start(wv[d][:], wvt[r, :])
                for d in range(DC):
                    r = slice(d * P, (d + 1) * P)
                    nc.sync.dma_start(qx[d][:], qt[r, :])
                    nc.sync.dma_start(wq[d][:], wqt[r, :])

                # ---- k projection: k^T[ck, s] ----
                ps = psA.tile([P, S], F32, tag="psA")
                for seg in range(2):
                    cs = slice(seg * SEG, (seg + 1) * SEG)
                    for d in range(DC):
                        nc.tensor.matmul(
                            ps[:, cs], wk[d][:], kx[d][:, cs],
                            start=(d == 0), stop=(d == DC - 1),
                        )
                nc.scalar.copy(kt_sb[:, 0, :], ps[0:HD, :])
                nc.scalar.copy(kt_sb[:, 1, :], ps[HD:P, :])

                # ---- v projection: v[s, cv] natural, into vaug ----
                for sc in range(SC):
                    ss = slice(sc * P, (sc + 1) * P)
                    pv = psA.tile([P, P], F32, tag="psA")
                    for d in range(DC):
                        nc.tensor.matmul(
                            pv[:], vx[d][:, ss], wv[d][:],
                            start=(d == 0), stop=(d == DC - 1),
                        )
                    for gl in range(GPC):
                        nc.scalar.copy(
                            vaug[:, sc, gl, 0:HD],
                            pv[:, gl * HD:(gl + 1) * HD],
                        )

                # ---- q projection: q^T[cq, s] ----
                for mq in range(CQ // P):
                    ms = slice(mq * P, (mq + 1) * P)
                    pq = psA.tile([P, S], F32, tag="psA")
                    for seg in range(2):
                        cs = slice(seg * SEG, (seg + 1) * SEG)
                        for d in range(DC):
                            nc.tensor.matmul(
                                pq[:, cs], wq[d][:, ms], qx[d][:, cs],
                                start=(d == 0), stop=(d == DC - 1),
                            )
                    nc.scalar.copy(qt_sb[:, 2 * mq, :], pq[0:HD, :])
                    nc.scalar.copy(qt_sb[:, 2 * mq + 1, :], pq[HD:P, :])

            # ---- w_o^T + bias loads (overlap with attention) ----
            wo = [rp.tile([P, CO], BF16, name=f"wo{d}") for d in range(DC)]
            for d in range(DC):
                nc.sync.dma_start(wo[d][:], wot[d * P:(d + 1) * P, :])

            # ---- attention: head pairs interleaved; AllGather per q-block ----
            nqb = CQ // P
            qb_per_ag = nqb // AG_CHUNKS
            agin = [dp.tile([qb_per_ag * P, S], BF16, name=f"agin{q}")
                    for q in range(AG_CHUNKS)]
            agout = [dp.tile([4 * qb_per_ag * P, S], BF16, name=f"agout{q}")
                     for q in range(AG_CHUNKS)]

            def scores_segs(m):
                nq0 = m * P
                if nq0 < SEG:
                    return [(nq0, SEG), (SEG, S)]
                return [(nq0, S)]

            with tc.tile_pool(name="probs", bufs=4) as pp:
                for pair in range(CQ // P):
                    heads = (2 * pair, 2 * pair + 1)
                    oas = {}
                    prs = {}
                    for m in range(SC):
                        for h in heads:
                            gl = h // HG
                            sc_ps = psA.tile([P, S], F32, tag="psA",
                                             name=f"sc{h}_{m}")
                            for (a, b2) in scores_segs(m):
                                nc.tensor.matmul(
                                    sc_ps[:, a:b2],
                                    kt_sb[:, gl, m * P:(m + 1) * P],
                                    qt_sb[:, h, a:b2],
                                    start=True, stop=True,
                                )
                            pr = pp.tile([P, S], BF16, tag="probs",
                                         name=f"pr{h}_{m}")
                            nc.scalar.activation(
                                pr[:, m * P:S], sc_ps[:, m * P:S], AF.Exp,
                                scale=1.0 / np.sqrt(HD),
                            )
                            nc.vector.tensor_mul(
                                pr[:, m * P:(m + 1) * P],
                                pr[:, m * P:(m + 1) * P], tri_sb[:]
                            )
                            prs[h] = pr
                        for h in heads:
                            gl = h // HG
                            if m == 0:
                                oas[h] = psB.tile([HD + 1, S], F32, tag="psB",
                                                  name=f"oa{h}")
                            for (a, b2) in scores_segs(m):
                                nc.tensor.matmul(
                                    oas[h][:, a:b2],
                                    vaug[:, m, gl, :],
                                    prs[h][:, a:b2],
                                    start=(m == 0),
                                    stop=(m == SC - 1) or (b2 == SEG and m == 3),
                                )
                    # normalize both heads of the pair, fire this q-block's AG
                    for h in heads:
                        qrow = h * HD
                        qpart = slice(qrow % P, qrow % P + HD)
                        # custom-DVE recip misreads PSUM at base partition 64
                        # on HW — stage the denominator row to SBUF first
                        den = pp.tile([1, S], F32, tag="den")
                        nc.scalar.copy(den[:], oas[h][HD:HD + 1, :])
                        rec = pp.tile([1, S], F32, tag="rec")
                        nc.vector.reciprocal_approx_fast(rec[:], den[:])
                        rbc = pp.tile([HD, S], F32, tag="rbc")
                        nc.gpsimd.partition_broadcast(rbc[:], rec[:])
                        nc.vector.tensor_mul(
                            attn_sb[qpart, pair, :], oas[h][0:HD, :], rbc[:]
                        )
                    ag_idx, ag_off = divmod(pair, qb_per_ag)
                    nc.sync.dma_start(
                        agin[ag_idx][ag_off * P:(ag_off + 1) * P, :],
                        attn_sb[:, pair, :],
                    )
                    if ag_off == qb_per_ag - 1:
                        nc.gpsimd.collective_compute(
                            "AllGather",
                            mybir.AluOpType.bypass,
                            replica_groups=[[0, 1, 2, 3], [4, 5, 6, 7]],
                            ins=[agin[ag_idx].opt()],
                            outs=[agout[ag_idx].opt()],
                        )

            # ---- o_proj: out[s, o] = attn_full^T.T @ w_o^T + b_o ----
            with tc.tile_pool(name="af", bufs=1) as ap_pool, \
                 tc.tile_pool(name="osb", bufs=3) as op:
                af = [ap_pool.tile([P, S], BF16, name=f"af{c}") for c in range(DC)]
                for c in range(DC):
                    r, q = divmod(c, nqb)
                    ag_idx, qo = divmod(q, qb_per_ag)
                    row = r * qb_per_ag * P + qo * P
                    nc.sync.dma_start(
                        af[c][:], agout[ag_idx][row:row + P, :]
                    )
                for sc in range(SC):
                    ss = slice(sc * P, (sc + 1) * P)
                    po = psB.tile([P, CO], F32, tag="psB")
                    nc.tensor.matmul(
                        po[:], ones_sb[:], bo_sb[:], start=True, stop=False,
                    )
                    for c in range(DC):
                        nc.tensor.matmul(
                            po[:], af[c][:, ss], wo[c][:],
                            start=False, stop=(c == DC - 1),
                        )
                    ot = op.tile([P, CO], F32, tag="osb")
                    nc.scalar.copy(ot[:], po[:])
                    nc.sync.dma_start(out[ss, :], ot[:])

    nc.compile()
    return nc


_nc_cache = None


def build_in_maps(inputs):
    Q = np.asarray(inputs["Q"], np.float32)
    K = np.asarray(inputs["K"], np.float32)
    V = np.asarray(inputs["V"], np.float32)
    w_q = np.asarray(inputs["w_q"], np.float32)
    w_k = np.asarray(inputs["w_k"], np.float32)
    w_v = np.asarray(inputs["w_v"], np.float32)
    w_o = np.asarray(inputs["w_o"], np.float32)
    b_o = np.asarray(inputs["b_o"], np.float32)

    bf = ml_dtypes.bfloat16
    tri = np.triu(np.ones((P, P), np.float32)).astype(bf)  # key i <= query j

    in_maps = []
    for c in range(NCORES):
        b, j = divmod(c, 4)
        in_maps.append({
            "qt": np.ascontiguousarray(Q[b].T).astype(bf),
            "kt": np.ascontiguousarray(K[b].T).astype(bf),
            "vt": np.ascontiguousarray(V[b].T).astype(bf),
            "wqt": np.ascontiguousarray(w_q[j * CQ:(j + 1) * CQ, :].T).astype(bf),
            "wkt": np.ascontiguousarray(w_k[j * CK:(j + 1) * CK, :].T).astype(bf),
            "wvt": np.ascontiguousarray(w_v[j * CK:(j + 1) * CK, :].T).astype(bf),
            "wot": np.ascontiguousarray(w_o[j * CO:(j + 1) * CO, :].T).astype(bf),
            "bo": b_o[None, j * CO:(j + 1) * CO].astype(bf),
            "tri": tri,
        })
    return in_maps


def kernel(**inputs):
    global _nc_cache
    in_maps = build_in_maps(inputs)
    if _nc_cache is None:
        _nc_cache = _build_nc()
    nc = _nc_cache

    trace = bool(int(os.environ.get("BASS_KERNEL_TRACE", "0")))
    res = bass_utils.run_bass_kernel_spmd(
        nc, in_maps, core_ids=list(range(NCORES)), trace=trace,
    )
    kernel.last_results = res

    out = np.empty((B, S, D), np.float32)
    for c in range(NCORES):
        b, j = divmod(c, 4)
        out[b][:, j * CO:(j + 1) * CO] = res.results[c]["out"]
    return out



# revision 9
# speedup vs baseline: 1.0320x; 1.0320x over previous
"""GQA (no RoPE) Trainium2 kernel, 8 NeuronCores — v2.

Sharding: 2 batches x 4 shards; each shard = 2 KV groups + 8 query heads.
Per core, heads are processed as 4 "pairsets" = (group0 head i, group1 head i)
with group-1 k/q resident at partitions 64:128, so the two 64-contract score
matmuls occupy disjoint PE row-groups and execute concurrently.

Key scheduling ideas (HAM clock gate: PE idle >3.4us drops it to 1.2GHz):
- q-projection of pairset p+1 and o-projection "waves" of pairset p-1 are
  interleaved into pairset p's attention, so the tensor queue never drains.
- AllGathers fire per half-row-block (cols 0:512 complete at key-block m=3),
  8 small AGs instead of 4 late ones; o_proj consumes each AG as a wave of
  short PSUM chains accumulated into SBUF f32 via DVE adds.
- softmax in transposed layout: denominators from an appended ones-column on
  v; reciprocal on DVE; partition-broadcast on gpsimd.

Self-contained: hardcodes B=2, S=1024, D=2048, G=8, HG=4, HD=64.
"""

import os
import sys

sys.path.insert(0, "/opt/trn_rl_repo")

import numpy as np
import ml_dtypes

import concourse.bass as bass
import concourse.mybir as mybir
import concourse.tile as tile
from concourse import bacc
from concourse import bass_utils

BF16 = mybir.dt.bfloat16
F32 = mybir.dt.float32
AF = mybir.ActivationFunctionType

B, S, D = 2, 1024, 2048
G, HG, HD = 8, 4, 64            # groups, heads/group, head dim
P = 128                          # partitions
NCORES = 8
GPC = 2                          # groups per core
NPS = 4                          # pairsets per core (one head per group each)
CQ = GPC * HG * HD               # q channels per core = 512
CK = GPC * HD                    # k/v channels per core = 128
CO = D // 4                      # output cols per core = 512
DC = D // P                      # contract chunks = 16
SC = S // P                      # seq chunks = 8
SEG = 512                        # psum bank width in f32


def _build_nc():
    nc = bacc.Bacc(
        "TRN2",
        target_bir_lowering=False,
        debug=False,
        enable_asserts=False,
        num_devices=NCORES,
    )

    # ---- I/O ----
    qt = nc.dram_tensor("qt", [D, S], BF16, kind="ExternalInput").ap()
    kt = nc.dram_tensor("kt", [D, S], BF16, kind="ExternalInput").ap()
    vt = nc.dram_tensor("vt", [D, S], BF16, kind="ExternalInput").ap()
    wqt = nc.dram_tensor("wqt", [D, CQ], BF16, kind="ExternalInput").ap()
    wkt = nc.dram_tensor("wkt", [D, CK], BF16, kind="ExternalInput").ap()
    wvt = nc.dram_tensor("wvt", [D, CK], BF16, kind="ExternalInput").ap()
    wot = nc.dram_tensor("wot", [D, CO], BF16, kind="ExternalInput").ap()
    bo = nc.dram_tensor("bo", [1, CO], BF16, kind="ExternalInput").ap()
    tri = nc.dram_tensor("tri", [P, P], BF16, kind="ExternalInput").ap()
    out = nc.dram_tensor("out", [S, CO], F32, kind="ExternalOutput").ap()

    with tile.TileContext(nc) as tc:
        with (
            tc.tile_pool(name="consts", bufs=1) as cp,
            tc.tile_pool(name="res", bufs=1) as rp,
            tc.tile_pool(name="psS", bufs=4, space="PSUM") as psS,
            tc.tile_pool(name="psO", bufs=4, space="PSUM") as psO,
            tc.tile_pool(name="dram", bufs=1, space="DRAM") as dp,
            tc.tile_pool(name="pr", bufs=5) as pp,
            tc.tile_pool(name="nrm", bufs=3) as npool,
            tc.tile_pool(name="af", bufs=5) as afp,
            tc.tile_pool(name="osb", bufs=2) as op,
            tc.tile_pool(name="xt", bufs=1) as xp,
        ):
            tri_sb = cp.tile([P, P], BF16)
            nc.sync.dma_start(tri_sb[:], tri[:])
            bo_sb = cp.tile([1, CO], BF16)
            nc.sync.dma_start(bo_sb[:], bo[:])
            ones_sb = cp.tile([1, P], BF16)
            nc.vector.memset(ones_sb[:], 1.0)

            # resident tensors
            kt2 = rp.tile([P, S], BF16)                 # k^T, g0 rows 0:64, g1 64:128
            qt2 = rp.tile([P, NPS, S], BF16)            # q^T per pairset, same split
            vaug = rp.tile([P, SC, GPC, HD + 1], BF16)  # v natural + ones col
            attn_sb = rp.tile([P, NPS, S], BF16)        # normalized attn^T
            osum = [rp.tile([P, CO], F32, name=f"osum{s}") for s in range(SC)]
            wo = [rp.tile([P, CO], BF16, name=f"wo{d}") for d in range(DC)]

            nc.vector.memset(vaug[:, :, :, HD:HD + 1], 1.0)

            # ---- input loads, ordered for earliest attention start ----
            kx = [xp.tile([P, S], BF16, name=f"kx{d}") for d in range(DC)]
            wk = [xp.tile([P, CK], BF16, name=f"wk{d}") for d in range(DC)]
            vx = [xp.tile([P, S], BF16, name=f"vx{d}") for d in range(DC)]
            wv = [xp.tile([P, CK], BF16, name=f"wv{d}") for d in range(DC)]
            qx = [xp.tile([P, S], BF16, name=f"qx{d}") for d in range(DC)]
            wq = [xp.tile([P, CQ], BF16, name=f"wq{d}") for d in range(DC)]
            for d in range(DC):
                r = slice(d * P, (d + 1) * P)
                nc.sync.dma_start(kx[d][:], kt[r, :])
                nc.sync.dma_start(wk[d][:], wkt[r, :])
            for d in range(DC):
                r = slice(d * P, (d + 1) * P)
                nc.sync.dma_start(vx[d][:], vt[r, :])
                nc.sync.dma_start(wv[d][:], wvt[r, :])
            for d in range(DC):
                r = slice(d * P, (d + 1) * P)
                nc.sync.dma_start(qx[d][:], qt[r, :])
            # wq block 0 first (attention pairset 0 needs only block 0)
            for blk in range(NPS):
                ms = slice(blk * P, (blk + 1) * P)
                for d in range(DC):
                    r = slice(d * P, (d + 1) * P)
                    nc.sync.dma_start(wq[d][:, ms], wqt[r, ms])
            for d in range(DC):
                nc.sync.dma_start(wo[d][:], wot[d * P:(d + 1) * P, :])

            # ---- k projection: k^T[ck, s], both groups stacked ----
            for seg in range(2):
                cs = slice(seg * SEG, (seg + 1) * SEG)
                ps = psS.tile([P, SEG], F32, tag="psS")
                for d in range(DC):
                    nc.tensor.matmul(
                        ps[:], wk[d][:], kx[d][:, cs],
                        start=(d == 0), stop=(d == DC - 1),
                    )
                nc.scalar.copy(kt2[:, cs], ps[:])

            # ---- v projection: v[s, cv] natural, into vaug ----
            for sc in range(SC):
                ss = slice(sc * P, (sc + 1) * P)
                pv = psS.tile([P, SEG], F32, tag="psS")
                for d in range(DC):
                    nc.tensor.matmul(
                        pv[:, 0:P], vx[d][:, ss], wv[d][:],
                        start=(d == 0), stop=(d == DC - 1),
                    )
                for gl in range(GPC):
                    nc.scalar.copy(
                        vaug[:, sc, gl, 0:HD],
                        pv[:, gl * HD:(gl + 1) * HD],
                    )

            def qproj(blk):
                ms = slice(blk * P, (blk + 1) * P)
                for seg in range(2):
                    cs = slice(seg * SEG, (seg + 1) * SEG)
                    pq = psS.tile([P, SEG], F32, tag="psS", name=f"q{blk}_{seg}")
                    for d in range(DC):
                        nc.tensor.matmul(
                            pq[:], wq[d][:, ms], qx[d][:, cs],
                            start=(d == 0), stop=(d == DC - 1),
                        )
                    nc.scalar.copy(qt2[:, blk, cs], pq[:])

            qproj(0)

            # ---- collectives: 8 half-AGs (pairset x col-half) ----
            agin = [dp.tile([P, SEG], BF16, name=f"agin{w}") for w in range(8)]
            agout = [dp.tile([4 * P, SEG], BF16, name=f"agout{w}")
                     for w in range(8)]

            def fire_ag(ps_idx, half):
                w = 2 * ps_idx + half
                cs = slice(half * SEG, (half + 1) * SEG)
                nc.sync.dma_start(agin[w][:], attn_sb[:, ps_idx, cs])
                nc.gpsimd.collective_compute(
                    "AllGather",
                    mybir.AluOpType.bypass,
                    replica_groups=[[0, 1, 2, 3], [4, 5, 6, 7]],
                    ins=[agin[w].opt()],
                    outs=[agout[w].opt()],
                )

            def normalize(ps_idx, half, oa):
                # oa = (oaA, oaB) psum tiles [HD+1, SEG] for this col-half
                cs = slice(half * SEG, (half + 1) * SEG)
                for x, base in ((0, 0), (1, HD)):
                    den = npool.tile([1, SEG], F32, tag="den")
                    nc.scalar.copy(den[:], oa[x][HD:HD + 1, :])
                    rec = npool.tile([1, SEG], F32, tag="rec")
                    nc.vector.reciprocal_approx_fast(rec[:], den[:])
                    rbc = npool.tile([HD, SEG], F32, tag="rbc")
                    nc.gpsimd.partition_broadcast(rbc[:], rec[:])
                    nc.vector.tensor_mul(
                        attn_sb[base:base + HD, ps_idx, cs],
                        oa[x][0:HD, :], rbc[:],
                    )
                fire_ag(ps_idx, half)

            # ---- o_proj waves ----
            # wave w = 2*src_pairset + half: contributes chunk (r, src_pairset)
            # for each source core r (0..3 within replica group) to output
            # rows half*512 ... +512.  Local chunk (r == my rank) comes from
            # attn_sb; remote from agout[w].  Accumulate into osum via DVE.
            waves_done = [0] * SC  # how many waves have hit each sc block

            def oproj_wave(ps_idx, half):
                w = 2 * ps_idx + half
                af = [None] * 4
                for r in range(4):
                    t = afp.tile([P, SEG], BF16, tag="af", name=f"af{w}_{r}")
                    nc.sync.dma_start(t[:], agout[w][r * P:(r + 1) * P, :])
                    af[r] = t
                for sci in range(4):
                    sc = half * 4 + sci
                    ss = slice(sci * P, (sci + 1) * P)
                    po = psS.tile([P, CO], F32, tag="psS", name=f"po{w}_{sc}")
                    first_wave = waves_done[sc] == 0
                    if first_wave:
                        nc.tensor.matmul(
                            po[:], ones_sb[:], bo_sb[:],
                            start=True, stop=False,
                        )
                    for r in range(4):
                        c = r * NPS + ps_idx
                        nc.tensor.matmul(
                            po[:], af[r][:, ss], wo[c][:],
                            start=(r == 0 and not first_wave),
                            stop=(r == 3),
                        )
                    if first_wave:
                        nc.vector.tensor_copy(osum[sc][:], po[:])
                    elif waves_done[sc] == NPS - 1:
                        ot = op.tile([P, CO], F32, tag="osb")
                        nc.vector.tensor_add(ot[:], po[:], osum[sc][:])
                        nc.sync.dma_start(out[sc * P:(sc + 1) * P, :], ot[:])
                    else:
                        nc.vector.tensor_add(osum[sc][:], po[:], osum[sc][:])
                    waves_done[sc] += 1

            # ---- attention: 4 pairsets, fillers interleaved ----
            for psx in range(NPS):
                # oa chains: [head x][col-half] -> psum [HD+1, SEG]
                oa = [[psO.tile([HD + 1, SEG], F32, tag="psO",
                                name=f"oa{psx}_{x}_{h}")
                       for h in range(2)] for x in range(2)]
                for m in range(SC):
                    m0 = m * P
                    if m0 < SEG:
                        regions = [(m0, SEG), (SEG, S)]
                    else:
                        regions = [(m0, S)]
                    prs = [[], []]
                    for x, base in ((0, 0), (1, HD)):
                        kb = kt2[base:base + HD, m0:m0 + P]
                        for (a, b2) in regions:
                            w = b2 - a
                            sc_ps = psS.tile([P, SEG], F32, tag="psS",
                                             name=f"sc{psx}_{x}_{m}_{a}")
                            nc.tensor.matmul(
                                sc_ps[:, 0:w], kb,
                                qt2[base:base + HD, psx, a:b2],
                                start=True, stop=True,
                            )
                            prx = pp.tile([P, SEG], BF16, tag="pr",
                                          name=f"pr{psx}_{x}_{m}_{a}")
                            nc.scalar.activation(
                                prx[:, 0:w], sc_ps[:, 0:w], AF.Exp,
                                scale=1.0 / np.sqrt(HD),
                            )
                            if a == m0:  # diagonal block: causal mask
                                nc.vector.tensor_mul(
                                    prx[:, 0:P], prx[:, 0:P], tri_sb[:]
                                )
                            prs[x].append((a, b2, prx))
                    for x in range(2):
                        for (a, b2, prx) in prs[x]:
                            half = 0 if a < SEG else 1
                            hb = half * SEG
                            nc.tensor.matmul(
                                oa[x][half][:, a - hb:b2 - hb],
                                vaug[:, m, x, :], prx[:, 0:b2 - a],
                                start=(m == 0),
                                stop=(m == 3 if half == 0 else m == SC - 1),
                            )
                    if m == 3:
                        normalize(psx, 0, (oa[0][0], oa[1][0]))
                        # fillers: q proj of next pairset, prev pairset's
                        # first o_proj wave
                        if psx < NPS - 1:
                            qproj(psx + 1)
                        if psx > 0:
                            oproj_wave(psx - 1, 0)
                normalize(psx, 1, (oa[0][1], oa[1][1]))
                if psx > 0:
                    oproj_wave(psx - 1, 1)
            oproj_wave(NPS - 1, 0)
            oproj_wave(NPS - 1, 1)

    nc.compile()
    return nc


_nc_cache = None


def build_in_maps(inputs):
    Q = np.asarray(inputs["Q"], np.float32)
    K = np.asarray(inputs["K"], np.float32)
    V = np.asarray(inputs["V"], np.float32)
    w_q = np.asarray(inputs["w_q"], np.float32)
    w_k = np.asarray(inputs["w_k"], np.float32)
    w_v = np.asarray(inputs["w_v"], np.float32)
    w_o = np.asarray(inputs["w_o"], np.float32)
    b_o = np.asarray(inputs["b_o"], np.float32)

    bf = ml_dtypes.bfloat16
    tri = np.triu(np.ones((P, P), np.float32)).astype(bf)  # key i <= query j

    # w_o contraction rows in global chunk-consumption order:
    # chunk (r, i) = source core r's pairset i = heads (8r+i, 8r+4+i)
    perm = []
    for r in range(4):
        for i in range(NPS):
            perm.extend(range(HD * (8 * r + i), HD * (8 * r + i) + HD))
            perm.extend(range(HD * (8 * r + 4 + i), HD * (8 * r + 4 + i) + HD))
    perm = np.array(perm)

    in_maps = []
    for c in range(NCORES):
        b, j = divmod(c, 4)
        # q columns for core j in pairset order: (g0 head i | g1 head i)
        qcols = []
        for i in range(NPS):
            qcols.extend(range(HD * (8 * j + i), HD * (8 * j + i) + HD))
            qcols.extend(range(HD * (8 * j + 4 + i), HD * (8 * j + 4 + i) + HD))
        qcols = np.array(qcols)
        in_maps.append({
            "qt": np.ascontiguousarray(Q[b].T).astype(bf),
            "kt": np.ascontiguousarray(K[b].T).astype(bf),
            "vt": np.ascontiguousarray(V[b].T).astype(bf),
            "wqt": np.ascontiguousarray(w_q[qcols, :].T).astype(bf),
            "wkt": np.ascontiguousarray(w_k[CK * j:CK * (j + 1), :].T).astype(bf),
            "wvt": np.ascontiguousarray(w_v[CK * j:CK * (j + 1), :].T).astype(bf),
            "wot": np.ascontiguousarray(
                w_o[CO * j:CO * (j + 1), :].T[perm, :]).astype(bf),
            "bo": b_o[None, CO * j:CO * (j + 1)].astype(bf),
            "tri": tri,
        })
    return in_maps


def kernel(**inputs):
    global _nc_cache
    in_maps = build_in_maps(inputs)
    if _nc_cache is None:
        _nc_cache = _build_nc()
    nc = _nc_cache

    trace = bool(int(os.environ.get("BASS_KERNEL_TRACE", "0")))
    res = bass_utils.run_bass_kernel_spmd(
        nc, in_maps, core_ids=list(range(NCORES)), trace=trace,
    )
    kernel.last_results = res

    out = np.empty((B, S, D), np.float32)
    for c in range(NCORES):
        b, j = divmod(c, 4)
        out[b][:, j * CO:(j + 1) * CO] = res.results[c]["out"]
    return out


# revision 14
# speedup vs baseline: 1.0701x; 1.0370x over previous
"""GQA (no RoPE) Trainium2 kernel, 8 NeuronCores — v3.

Sharding: 2 batches x 4 shards; each shard = 2 KV groups + 8 query heads.
Heads processed as 4 "pairsets" = (group0 head i, group1 head i) with group-1
k/q resident at partitions 64:128, so the two 64-contract score matmuls occupy
disjoint PE row-groups and execute concurrently.

Schedule (HAM clock gate: PE idle >3.4us drops the clock to 1.2GHz; the CC
engine runs collectives serially and collective_compute head-of-line blocks
the gpsimd queue):
- all projections run up front, paced by the input DMA stream;
- attention pairsets are scalar(exp)-paced; softmax normalize uses a
  partition-broadcast access pattern on DVE (nothing but the 4 AllGathers
  ever enters the gpsimd queue);
- one AllGather per pairset, fired at pairset end; o_proj of pairset p runs
  as a "wave" at the end of pairset p+1 (its AG has completed by then),
  accumulating into SBUF f32 via DVE so no PSUM bank is held across waves.

Self-contained: hardcodes B=2, S=1024, D=2048, G=8, HG=4, HD=64.
"""

import os
import sys

sys.path.insert(0, "/opt/trn_rl_repo")

import numpy as np
import ml_dtypes

import concourse.bass as bass
import concourse.mybir as mybir
import concourse.tile as tile
from concourse import bacc
from concourse import bass_utils

BF16 = mybir.dt.bfloat16
F32 = mybir.dt.float32
AF = mybir.ActivationFunctionType

B, S, D = 2, 1024, 2048
G, HG, HD = 8, 4, 64
P = 128
NCORES = 8
GPC = 2
NPS = 4                          # pairsets per core
CQ = GPC * HG * HD               # 512
CK = GPC * HD                    # 128
CO = D // 4                      # 512
DC = D // P                      # 16
SC = S // P                      # 8
SEG = 512


def _build_nc():
    nc = bacc.Bacc(
        "TRN2",
        target_bir_lowering=False,
        debug=False,
        enable_asserts=False,
        num_devices=NCORES,
    )

    qt = nc.dram_tensor("qt", [D, S], BF16, kind="ExternalInput").ap()
    kt = nc.dram_tensor("kt", [D, S], BF16, kind="ExternalInput").ap()
    vt = nc.dram_tensor("vt", [D, S], BF16, kind="ExternalInput").ap()
    wqt = nc.dram_tensor("wqt", [D, CQ], BF16, kind="ExternalInput").ap()
    wkt = nc.dram_tensor("wkt", [D, CK], BF16, kind="ExternalInput").ap()
    wvt = nc.dram_tensor("wvt", [D, CK], BF16, kind="ExternalInput").ap()
    wot = nc.dram_tensor("wot", [D, CO], BF16, kind="ExternalInput").ap()
    bo = nc.dram_tensor("bo", [1, CO], BF16, kind="ExternalInput").ap()
    tri = nc.dram_tensor("tri", [P, P], BF16, kind="ExternalInput").ap()
    out = nc.dram_tensor("out", [S, CO], F32, kind="ExternalOutput").ap()

    with tile.TileContext(nc) as tc:
        with (
            tc.tile_pool(name="consts", bufs=1) as cp,
            tc.tile_pool(name="res", bufs=1) as rp,
            tc.tile_pool(name="psS", bufs=4, space="PSUM") as psS,
            tc.tile_pool(name="psO", bufs=4, space="PSUM") as psO,
            tc.tile_pool(name="dram", bufs=1, space="DRAM") as dp,
            tc.tile_pool(name="pr", bufs=5) as pp,
            tc.tile_pool(name="nrm", bufs=2) as npool,
            tc.tile_pool(name="af", bufs=4) as afp,
            tc.tile_pool(name="osb", bufs=2) as op,
            tc.tile_pool(name="xt", bufs=1) as xp,
        ):
            tri_sb = cp.tile([P, P], BF16)
            nc.sync.dma_start(tri_sb[:], tri[:])
            bo_sb = cp.tile([1, CO], BF16)
            nc.sync.dma_start(bo_sb[:], bo[:])
            ones_sb = cp.tile([1, P], BF16)
            nc.vector.memset(ones_sb[:], 1.0)
            ones64f = cp.tile([1, HD], F32)
            nc.vector.memset(ones64f[:], 1.0)

            kt2 = rp.tile([P, S], BF16)
            qt2 = rp.tile([P, NPS, S], BF16)
            vaug = rp.tile([P, SC, GPC, HD + 1], BF16)
            attn_sb = rp.tile([P, NPS, S], BF16)
            osum = [rp.tile([P, CO], F32, name=f"osum{s}") for s in range(SC)]
            wo = [rp.tile([P, CO], BF16, name=f"wo{d}") for d in range(DC)]

            nc.vector.memset(vaug[:, :, :, HD:HD + 1], 1.0)

            # ---- input loads: k first, q+wq next, v, then w_o ----
            kx = [xp.tile([P, S], BF16, name=f"kx{d}") for d in range(DC)]
            wk = [xp.tile([P, CK], BF16, name=f"wk{d}") for d in range(DC)]
            vx = [xp.tile([P, S], BF16, name=f"vx{d}") for d in range(DC)]
            wv = [xp.tile([P, CK], BF16, name=f"wv{d}") for d in range(DC)]
            qx = [xp.tile([P, S], BF16, name=f"qx{d}") for d in range(DC)]
            wq = [xp.tile([P, CQ], BF16, name=f"wq{d}") for d in range(DC)]
            for d in range(DC):
                r = slice(d * P, (d + 1) * P)
                nc.sync.dma_start(kx[d][:], kt[r, :])
                nc.sync.dma_start(wk[d][:], wkt[r, :])
            for d in range(DC):
                r = slice(d * P, (d + 1) * P)
                nc.sync.dma_start(qx[d][:], qt[r, :])
                nc.sync.dma_start(wq[d][:], wqt[r, :])
            for d in range(DC):
                r = slice(d * P, (d + 1) * P)
                nc.sync.dma_start(vx[d][:], vt[r, :])
                nc.sync.dma_start(wv[d][:], wvt[r, :])
            for d in range(DC):
                nc.sync.dma_start(wo[d][:], wot[d * P:(d + 1) * P, :])

            # ---- projections, all up front ----
            for seg in range(2):
                cs = slice(seg * SEG, (seg + 1) * SEG)
                ps = psS.tile([P, SEG], F32, tag="psS", name=f"kp{seg}")
                for d in range(DC):
                    nc.tensor.matmul(
                        ps[:], wk[d][:], kx[d][:, cs],
                        start=(d == 0), stop=(d == DC - 1),
                    )
                nc.scalar.copy(kt2[:, cs], ps[:])

            for blk in range(NPS):
                ms = slice(blk * P, (blk + 1) * P)
                for seg in range(2):
                    cs = slice(seg * SEG, (seg + 1) * SEG)
                    pq = psS.tile([P, SEG], F32, tag="psS", name=f"qp{blk}_{seg}")
                    for d in range(DC):
                        nc.tensor.matmul(
                            pq[:], wq[d][:, ms], qx[d][:, cs],
                            start=(d == 0), stop=(d == DC - 1),
                        )
                    nc.vector.tensor_copy(qt2[:, blk, cs], pq[:])

            for sc in range(SC):
                ss = slice(sc * P, (sc + 1) * P)
                pv = psS.tile([P, SEG], F32, tag="psS", name=f"vp{sc}")
                for d in range(DC):
                    nc.tensor.matmul(
                        pv[:, 0:P], vx[d][:, ss], wv[d][:],
                        start=(d == 0), stop=(d == DC - 1),
                    )
                for gl in range(GPC):
                    nc.vector.tensor_copy(
                        vaug[:, sc, gl, 0:HD],
                        pv[:, gl * HD:(gl + 1) * HD],
                    )

            # ---- collectives ----
            agin = [dp.tile([P, S], BF16, name=f"agin{w}") for w in range(NPS)]
            agout = [dp.tile([4 * P, S], BF16, name=f"agout{w}")
                     for w in range(NPS)]

            def fire_ag(ps_idx):
                nc.sync.dma_start(agin[ps_idx][:], attn_sb[:, ps_idx, :])
                nc.gpsimd.collective_compute(
                    "AllGather",
                    mybir.AluOpType.bypass,
                    replica_groups=[[0, 1, 2, 3], [4, 5, 6, 7]],
                    ins=[agin[ps_idx].opt()],
                    outs=[agout[ps_idx].opt()],
                )

            def normalize(ps_idx, half, oa_pair):
                # denominators -> reciprocals -> broadcast across partitions
                # via two concurrent rank-1 col-tiled matmuls (DVE rejects
                # partition-stride-0 APs; gpsimd must stay AG-only)
                cs = slice(half * SEG, (half + 1) * SEG)
                recs = []
                for x in range(2):
                    den = npool.tile([1, SEG], F32, tag="den")
                    nc.scalar.copy(den[:], oa_pair[x][HD:HD + 1, :])
                    rec = npool.tile([1, SEG], F32, tag="rec")
                    nc.vector.reciprocal_approx_fast(rec[:], den[:])
                    recs.append(rec)
                rb_ps = psS.tile([P, SEG], F32, tag="psS",
                                 name=f"rb{ps_idx}_{half}")
                nc.tensor.matmul(rb_ps[0:HD, :], ones64f[:], recs[0][:],
                                 start=True, stop=True)
                nc.tensor.matmul(rb_ps[HD:P, :], ones64f[:], recs[1][:],
                                 start=True, stop=True, skip_group_check=True)
                rb_sb = npool.tile([P, SEG], F32, tag="rbc")
                nc.vector.tensor_copy(rb_sb[:], rb_ps[:])
                for x, base in ((0, 0), (1, HD)):
                    nc.vector.tensor_mul(
                        attn_sb[base:base + HD, ps_idx, cs],
                        oa_pair[x][0:HD, :],
                        rb_sb[base:base + HD, :],
                    )

            waves_done = [0] * SC

            def oproj_wave(ps_idx):
                af = []
                for r in range(4):
                    t = afp.tile([P, S], BF16, tag="af", name=f"af{ps_idx}_{r}")
                    nc.sync.dma_start(t[:], agout[ps_idx][r * P:(r + 1) * P, :])
                    af.append(t)
                for sc in range(SC):
                    ss = slice(sc * P, (sc + 1) * P)
                    po = psS.tile([P, CO], F32, tag="psS", name=f"po{ps_idx}_{sc}")
                    first = waves_done[sc] == 0
                    if first:
                        nc.tensor.matmul(
                            po[:], ones_sb[:], bo_sb[:], start=True, stop=False,
                        )
                    for r in range(4):
                        c = r * NPS + ps_idx
                        nc.tensor.matmul(
                            po[:], af[r][:, ss], wo[c][:],
                            start=(r == 0 and not first),
                            stop=(r == 3),
                        )
                    if first:
                        nc.vector.tensor_copy(osum[sc][:], po[:])
                    elif waves_done[sc] == NPS - 1:
                        ot = op.tile([P, CO], F32, tag="osb")
                        nc.vector.tensor_add(ot[:], po[:], osum[sc][:])
                        nc.sync.dma_start(out[sc * P:(sc + 1) * P, :], ot[:])
                    else:
                        nc.vector.tensor_add(osum[sc][:], po[:], osum[sc][:])
                    waves_done[sc] += 1

            # ---- attention ----
            for psx in range(NPS):
                oa = [[psO.tile([HD + 1, SEG], F32, tag="psO",
                                name=f"oa{psx}_{x}_{h}")
                       for h in range(2)] for x in range(2)]
                for m in range(SC):
                    m0 = m * P
                    regions = [(m0, SEG), (SEG, S)] if m0 < SEG else [(m0, S)]
                    prs = [[], []]
                    for x, base in ((0, 0), (1, HD)):
                        kb = kt2[base:base + HD, m0:m0 + P]
                        for (a, b2) in regions:
                            w = b2 - a
                            sc_ps = psS.tile([P, SEG], F32, tag="psS",
                                             name=f"sc{psx}_{x}_{m}_{a}")
                            nc.tensor.matmul(
                                sc_ps[:, 0:w], kb,
                                qt2[base:base + HD, psx, a:b2],
                                start=True, stop=True,
                            )
                            prx = pp.tile([P, SEG], BF16, tag="pr",
                                          name=f"pr{psx}_{x}_{m}_{a}")
                            nc.scalar.activation(
                                prx[:, 0:w], sc_ps[:, 0:w], AF.Exp,
                                scale=1.0 / np.sqrt(HD),
                            )
                            if a == m0:
                                nc.vector.tensor_mul(
                                    prx[:, 0:P], prx[:, 0:P], tri_sb[:]
                                )
                            prs[x].append((a, b2, prx))
                    for x in range(2):
                        for (a, b2, prx) in prs[x]:
                            half = 0 if a < SEG else 1
                            hb = half * SEG
                            nc.tensor.matmul(
                                oa[x][half][:, a - hb:b2 - hb],
                                vaug[:, m, x, :], prx[:, 0:b2 - a],
                                start=(m == 0),
                                stop=(m == 3 if half == 0 else m == SC - 1),
                            )
                    if m == 3:
                        normalize(psx, 0, (oa[0][0], oa[1][0]))
                normalize(psx, 1, (oa[0][1], oa[1][1]))
                fire_ag(psx)
                if psx > 0:
                    oproj_wave(psx - 1)
            oproj_wave(NPS - 1)

    nc.compile()
    return nc


_nc_cache = None


def build_in_maps(inputs):
    Q = np.asarray(inputs["Q"], np.float32)
    K = np.asarray(inputs["K"], np.float32)
    V = np.asarray(inputs["V"], np.float32)
    w_q = np.asarray(inputs["w_q"], np.float32)
    w_k = np.asarray(inputs["w_k"], np.float32)
    w_v = np.asarray(inputs["w_v"], np.float32)
    w_o = np.asarray(inputs["w_o"], np.float32)
    b_o = np.asarray(inputs["b_o"], np.float32)

    bf = ml_dtypes.bfloat16
    tri = np.triu(np.ones((P, P), np.float32)).astype(bf)

    # w_o contraction rows in chunk order: chunk (r, i) = core r's pairset i
    # = heads (8r+i, 8r+4+i)
    perm = []
    for r in range(4):
        for i in range(NPS):
            perm.extend(range(HD * (8 * r + i), HD * (8 * r + i) + HD))
            perm.extend(range(HD * (8 * r + 4 + i), HD * (8 * r + 4 + i) + HD))
    perm = np.array(perm)

    in_maps = []
    for c in range(NCORES):
        b, j = divmod(c, 4)
        qcols = []
        for i in range(NPS):
            qcols.extend(range(HD * (8 * j + i), HD * (8 * j + i) + HD))
            qcols.extend(range(HD * (8 * j + 4 + i), HD * (8 * j + 4 + i) + HD))
        qcols = np.array(qcols)
        in_maps.append({
            "qt": np.ascontiguousarray(Q[b].T).astype(bf),
            "kt": np.ascontiguousarray(K[b].T).astype(bf),
            "vt": np.ascontiguousarray(V[b].T).astype(bf),
            "wqt": np.ascontiguousarray(w_q[qcols, :].T).astype(bf),
            "wkt": np.ascontiguousarray(w_k[CK * j:CK * (j + 1), :].T).astype(bf),
            "wvt": np.ascontiguousarray(w_v[CK * j:CK * (j + 1), :].T).astype(bf),
            "wot": np.ascontiguousarray(
                w_o[CO * j:CO * (j + 1), :].T[perm, :]).astype(bf),
            "bo": b_o[None, CO * j:CO * (j + 1)].astype(bf),
            "tri": tri,
        })
    return in_maps


def kernel(**inputs):
    global _nc_cache
    in_maps = build_in_maps(inputs)
    if _nc_cache is None:
        _nc_cache = _build_nc()
    nc = _nc_cache

    trace = bool(int(os.environ.get("BASS_KERNEL_TRACE", "0")))
    res = bass_utils.run_bass_kernel_spmd(
        nc, in_maps, core_ids=list(range(NCORES)), trace=trace,
    )
    kernel.last_results = res

    out = np.empty((B, S, D), np.float32)
    for c in range(NCORES):
        b, j = divmod(c, 4)
        out[b][:, j * CO:(j + 1) * CO] = res.results[c]["out"]
    return out
